# revision 19
# baseline (speedup 1.0000x reference)
"""GRU-style segmented-scan aggregator for Trainium2 (8 NeuronCores).

Reference computes, per node n with messages m_1..m_L sorted by time t:
    h <- W @ (m + h) + b   starting from h = 0
and returns the final h per node (zeros for empty nodes).

Because every step uses the SAME matrix W, the final state has the closed
form (h_0 = 0):
    h = sum_{k=0}^{L-1} W^{k+1} m_{(L-1-k)}  +  S_L b,   S_L = sum_{p<L} W^p
i.e. the k-th message FROM THE END is hit by W^{k+1}.  This turns the
sequential scan into independent batched matmuls against precomputed powers
of W -- ideal for the PE array.

Tricks that shrink the device work:
  * Two-level blocking (B = 16): split k = q*B + r, so W^{k+1} = W^{qB} W^{r+1}.
    Each device accumulates block partials c_q with only W^1..W^B streamed,
    then folds the q=1 block with one extra matmul h += W^B c_1.  Messages at
    k >= 2B (~0.02%) fold into the host term.  The q=1 step that needs W^B
    is hoisted to stream block B-2 (which carries a copy of W^B), so c_1 is
    complete one block early and its bf16 drain overlaps the last block.
  * Bias fold: h = sum W^{k+1} m'_k with m'_last = m_last + W^{-1} S_L b,
    so no bias pass exists on device (safe here: ||W^{-1} S_L b|| <= ~4).

Everything streams in bf16 (1 PE cycle/column vs 4 for fp32; half the HBM
bytes), accumulates in fp32 PSUM.

Per-core schedule (SPMD over 8 cores; nodes sorted by message count desc and
dealt round-robin, <= 1024 slots/core):
  * PSUM: per 128-feature chunk i, accumulators p0a (slots 0-511), p0b
    (slots 512-1023), p1 (q=1 block, <=512 slots).  One bank each.
  * step r: lhsT = W^{r+1}; rhs columns = r-th-from-block-end messages.
    n0_r / n1_r shrink as segments exhaust -- ~0% padding.
  * stream blocks are CONTIGUOUS dram tensors; two DMA paths run ahead of
    the PE freely (zero SBUF reuse, so DMA never blocks on compute):
      - hi_r = [w_{r+1} | q1 msgs | q0 feature-hi] on the sync HWDGE queue
        (lane-credit recycling waits are legal single waits for loads),
      - q0 feature-lo packed into 6 SWDGE super-blocks, which leaves the
        2 output stores fresh SWDGE lanes (a recycled lane would need
        credit + data = 2 waits; this walrus build allows only one).
  * tail: c_1 -> bf16 (DVE) during block B-1; outer matmul W^B c_1; PSUM
    drains run as two per-chunk chains (chunk0 on DVE, chunk1 on ACT) into
    one [128, 1024] tile each, stored by one SWDGE DMA each.

Host does the (cheap) marshalling: lexsort by (index, t), gather into the
(r, q)-major column layout, precompute W powers in fp64, scatter back.
"""

import numpy as np
import ml_dtypes

import concourse.bass as bass
import concourse.mybir as mybir
from concourse import tile
from concourse.bass_utils import run_bass_kernel_spmd
import bass_rust

_N_PROCS = 27

BF16 = ml_dtypes.bfloat16


class _SplitDrainTC(tile.TileContext):
    """TileContext whose kernel-tail drain is split into one drain per proc.

    The walrus build in this container rejects instructions carrying more
    than one sync wait; the stock tail drain waits on every proc at once.
    Emitting a chain of drains, each waiting on a single semaphore, is
    semantically identical (all procs quiesced before the exit barrier).
    """

    def _drain_and_barrier(self, tick_clock, wait_clock):
        gc = tick_clock.global_clock
        for p in range(_N_PROCS):
            if gc[p] <= 0:
                continue
            d = self.nc.sync.drain()
            vc = bass_rust.VectorClock(
                [gc[q] if q == p else 0 for q in range(_N_PROCS)])
            wait_clock.add_sem_waits(d.ins, bass_rust.ScopedClock({None: vc}))
        assert self.sems is not None
        popped = self.nc._tile_sem_poison_stack.pop()
        assert popped is self._sem_poison
        # One barrier, no per-sem clear: the compiler's kernel epilogue
        # zeroes every semaphore after this block anyway, so the stock
        # clear_and_free + second barrier only add exit latency.
        self.nc.all_engine_barrier()

N_CORES = 8
DIM = 256
SLOTS = 1024  # node slots per core
BLK = 16      # power-blocking factor B


def _q1_steps(R1):
    """q=1 steps per stream block: step r at block r, except a step at
    block BLK-1 is hoisted to block BLK-2 (with a W^B weight copy) so c_1
    finishes a block early."""
    steps = [[] for _ in range(BLK)]
    for s in range(R1):
        steps[min(s, BLK - 2)].append(s)
    return steps


N_LO_SWDGE = 5  # lo blocks issued on SWDGE (5 + 2 stores = 7 of 8 fresh lanes)


_NC_CACHE: dict = {}


def _build_nc(n0: tuple, n1: tuple):
    """Build the Bass program for one core (shared by all 8 via SPMD)."""
    f32 = mybir.dt.float32
    bf16 = mybir.dt.bfloat16
    nc = bass.Bass()

    R0 = len(n0)
    R1 = len(n1)
    nq1 = n1[0] if R1 else 0
    rb_last = max((r for r in range(R0) if n0[r] > 512), default=-1)
    q1s = _q1_steps(R1)

    def hi_width(r):
        w = 512 + 2 * sum(n1[s] for s in q1s[r])
        if any(s == BLK - 1 for s in q1s[r]):
            w += 512                       # W^B copy for the hoisted step
        return w

    wq0 = nc.dram_tensor("wq0", [128, hi_width(0)], bf16,
                         kind="ExternalInput")
    his = [nc.dram_tensor(f"hi{r}",
                          [128, (hi_width(r) if r else 0) + n0[r]],
                          bf16, kind="ExternalInput") for r in range(R0)]
    los = [nc.dram_tensor(f"lo{r}", [128, n0[r]], bf16, kind="ExternalInput")
           for r in range(R0)]
    outs = [nc.dram_tensor(f"o{i}", [128, SLOTS], bf16, kind="ExternalOutput")
            for i in range(2)]

    with _SplitDrainTC(nc) as tc:
        with (
            tc.tile_pool(name="m", bufs=1) as mpool,
            tc.tile_pool(name="misc", bufs=1) as miscpool,
            tc.tile_pool(name="ps", bufs=1, space="PSUM") as pspool,
        ):
            p0a = [pspool.tile([128, 512], f32, tag=f"p0a{i}", name=f"p0a{i}")
                   for i in range(2)]
            p0b = [pspool.tile([128, 512], f32, tag=f"p0b{i}", name=f"p0b{i}")
                   for i in range(2)]
            p1 = [pspool.tile([128, 512], f32, tag=f"p1{j}", name=f"p1{j}")
                  for j in range(2)] if R1 else None

            wblk = None          # tile+offset holding W^B for the outer fold
            for r in range(R0):
                a = n0[r]
                hw = hi_width(r)
                if r == 0:
                    tw = mpool.tile([128, hw], bf16, tag="tw0", name="tw0")
                    nc.sync.dma_start(tw[:], wq0[:])
                    th = mpool.tile([128, a], bf16, tag="th0", name="th0")
                    nc.sync.dma_start(th[:], his[0][:])
                    hoff = 0
                else:
                    tw = th = mpool.tile([128, hw + a], bf16,
                                         tag=f"th{r}", name=f"th{r}")
                    nc.sync.dma_start(th[:], his[r][:])
                    hoff = hw
                tg = mpool.tile([128, a], bf16, tag=f"tl{r}", name=f"tl{r}")
                goff = 0
                # first few lo blocks ride the (otherwise idle) SWDGE path so
                # the cold-start feed keeps up; the rest stay on the sync
                # queue in need order
                if r < N_LO_SWDGE:
                    nc.gpsimd.dma_start(tg[:], los[r][:])
                else:
                    nc.sync.dma_start(tg[:], los[r][:])
                if r == BLK - 1 and R1 and wblk is None:
                    wblk = (tw, 0)
                # q1 message offsets within tw
                moff = 512 + (512 if any(s == BLK - 1 for s in q1s[r]) else 0)
                q1list = []
                for s in (q1s[r] if r < len(q1s) else []):
                    woff = 512 if s == BLK - 1 else 0
                    if s == BLK - 1:
                        wblk = (tw, 512)
                    q1list.append((s, n1[s], woff, moff))
                    moff += 2 * n1[s]
                for i in range(2):          # output feature chunk
                    for j in range(2):      # contraction chunk
                        wt = tw[:, j * 256 + i * 128: j * 256 + (i + 1) * 128]
                        src, boff = (th, hoff) if j == 0 else (tg, goff)
                        nc.tensor.matmul(
                            p0a[i][:, 0:min(a, 512)], wt,
                            src[:, boff: boff + min(a, 512)],
                            start=(r == 0 and j == 0), stop=False,
                            skip_group_check=True,
                        )
                        if a > 512:
                            nc.tensor.matmul(
                                p0b[i][:, 0:a - 512], wt,
                                src[:, boff + 512: boff + a],
                                start=(r == 0 and j == 0),
                                stop=(r == rb_last and i == 1 and j == 1),
                                skip_group_check=True,
                            )
                        for s, cs, woff, mo in q1list:
                            wts = tw[:, woff + j * 256 + i * 128:
                                     woff + j * 256 + (i + 1) * 128]
                            nc.tensor.matmul(
                                p1[i][:, 0:cs], wts,
                                tw[:, mo + j * cs: mo + (j + 1) * cs],
                                start=(r == 0 and j == 0), stop=False,
                                skip_group_check=True,
                            )
                # c_1 -> bf16 right after its last inner step (block B-2):
                # the DVE drain overlaps block B-1 on the PE
                if R1 and r == BLK - 2:
                    c1 = []
                    for j in range(2):
                        t = miscpool.tile([128, nq1], bf16, tag=f"c1{j}",
                                          name=f"c1{j}")
                        nc.vector.tensor_copy(t[:], p1[j][:, 0:nq1])
                        c1.append(t)

            if R1 and R0 < BLK:              # no hoist happened; drain now
                c1 = []
                for j in range(2):
                    t = miscpool.tile([128, nq1], bf16, tag=f"c1{j}",
                                      name=f"c1{j}")
                    nc.vector.tensor_copy(t[:], p1[j][:, 0:nq1])
                    c1.append(t)

            # outer fold q=1:  p0a += (W^B).T.T @ c1
            if R1:
                assert wblk is not None
                wtile, wo = wblk
                for i in range(2):
                    for j in range(2):
                        wt = wtile[:, wo + j * 256 + i * 128:
                                   wo + j * 256 + (i + 1) * 128]
                        nc.tensor.matmul(
                            p0a[i][:, 0:nq1], wt, c1[j][:],
                            start=False, stop=(i == 1 and j == 1),
                            skip_group_check=True,
                        )

            # drain PSUM -> bf16 SBUF as two per-chunk chains (chunk 0 on
            # DVE, chunk 1 on ACT) into one tile each, then one SWDGE store
            # per chunk (fresh DMASW lanes -> single producer wait)
            ot = [miscpool.tile([128, SLOTS], bf16, tag=f"ot{i}", name=f"ot{i}")
                  for i in range(2)]
            nc.vector.tensor_copy(ot[0][:, 512:1024], p0b[0][:])
            nc.scalar.copy(ot[1][:, 512:1024], p0b[1][:])
            nc.vector.tensor_copy(ot[0][:, 0:512], p0a[0][:])
            nc.scalar.copy(ot[1][:, 0:512], p0a[1][:])
            for i in range(2):
                nc.gpsimd.dma_start(outs[i][:], ot[i][:])
    return nc


def _prepare(msg, index, t, dim_size, W, b):
    """Host-side marshalling. Returns (in_maps, node_ids, schedule key)."""
    E, D = msg.shape
    counts = np.bincount(index, minlength=dim_size)
    order = np.lexsort((t, index))            # stable: primary index, secondary t
    msg_sorted = msg[order]                   # [E, D] grouped by node, t-ascending
    seg_starts = np.zeros(dim_size, np.int64)
    seg_starts[1:] = np.cumsum(counts)[:-1]

    nodesort = np.argsort(-counts, kind="stable")
    nz = nodesort[counts[nodesort] > 0]
    per_core = -(-len(nz) // N_CORES)
    assert per_core <= SLOTS, f"too many nodes per core: {per_core}"

    node_ids = np.full((N_CORES, SLOTS), -1, np.int64)
    for c in range(N_CORES):
        ids = nz[c::N_CORES]
        node_ids[c, :len(ids)] = ids
    cc = np.where(node_ids >= 0, counts[np.maximum(node_ids, 0)], 0)  # [8, SLOTS]

    Lmax = int(cc.max())
    K0 = min(Lmax, 2 * BLK)
    R0 = min(BLK, K0)
    R1 = max(0, K0 - BLK)
    n0 = tuple(int((cc > r).sum(axis=1).max()) for r in range(R0))
    n1 = tuple(int((cc > BLK + r).sum(axis=1).max()) for r in range(R1))
    assert (not n1) or n1[0] <= 512, f"q1 region too wide: {n1[0]}"
    q1s = _q1_steps(R1)

    def slot_pos(qb, r, nslots):
        nid = node_ids[:, :nslots]
        ck = cc[:, :nslots]
        active = (qb + r) < ck
        pos = seg_starts[np.maximum(nid, 0)] + ck - 1 - (qb + r)
        return np.where(active, pos, -1)

    # weights: powers of W in fp64, stored transposed (lhsT chunks)
    Wd = W.astype(np.float64)
    bd = b.astype(np.float64)
    wfull = np.empty((128, R0 * 512), np.float64)
    s_table = np.zeros((Lmax + 1, D), np.float64)   # s_p = S_p b
    Wpows = []                                      # W^{k+1}, k = 0..Lmax-1
    P = Wd.copy()
    for k in range(Lmax):
        if k < R0:
            WT = P.T
            wfull[:, k * 512:k * 512 + 256] = WT[:128, :]
            wfull[:, k * 512 + 256:(k + 1) * 512] = WT[128:, :]
        Wpows.append(P)
        s_table[k + 1] = Wd @ s_table[k] + bd
        P = P @ Wd
    wfull16 = wfull.astype(BF16)

    # bias fold into the last message (+ host-folded k >= K0 tail)
    WiS = np.linalg.solve(Wd, s_table.T).T          # [Lmax+1, 256]
    fold = WiS[cc]                                   # [8, SLOTS, 256] fp64
    for k in range(K0, Lmax):
        act = k < cc
        cs, ss = np.nonzero(act)
        pos = seg_starts[node_ids[cs, ss]] + cc[cs, ss] - 1 - k
        Wk = Wpows[k - 1] if k >= 1 else np.eye(D)   # W^k
        fold[cs, ss] += msg_sorted[pos].astype(np.float64) @ Wk.T

    def gathered16(qb, r, nslots, c):
        ri = slot_pos(qb, r, nslots)[c]
        if qb == 0 and r == 0:
            Mg = (msg_sorted[np.maximum(ri, 0)].astype(np.float64)
                  + fold[c, :nslots]).astype(BF16)
        else:
            Mg = msg16[np.maximum(ri, 0)]
        Mg[ri < 0] = 0.0
        return Mg                                     # [nslots, 256]

    msg16 = msg_sorted.astype(BF16)
    in_maps = [dict() for _ in range(N_CORES)]
    for r in range(R0):
        a = n0[r]
        hoisted = any(s == BLK - 1 for s in q1s[r])
        hw = 512 + (512 if hoisted else 0) + 2 * sum(n1[s] for s in q1s[r])
        for c in range(N_CORES):
            blk = np.zeros((128, hw + a), BF16)
            blk[:, 0:512] = wfull16[:, r * 512:(r + 1) * 512]
            mo = 512
            if hoisted:
                blk[:, 512:1024] = wfull16[:, (BLK - 1) * 512:BLK * 512]
                mo = 1024
            for s in q1s[r]:
                cs = n1[s]
                Mg1 = gathered16(BLK, s, cs, c)
                blk[:, mo:mo + cs] = Mg1[:, :128].T
                blk[:, mo + cs:mo + 2 * cs] = Mg1[:, 128:].T
                mo += 2 * cs
            Mg = gathered16(0, r, a, c)
            blk[:, hw:] = Mg[:, :128].T
            if r == 0:
                in_maps[c]["wq0"] = np.ascontiguousarray(blk[:, :hw])
                in_maps[c]["hi0"] = np.ascontiguousarray(blk[:, hw:])
            else:
                in_maps[c][f"hi{r}"] = blk
            in_maps[c][f"lo{r}"] = np.ascontiguousarray(Mg[:, 128:].T)
    return in_maps, node_ids, (n0, n1)


def _run(inputs: dict, trace: bool = False, **run_kwargs):
    msg = np.ascontiguousarray(np.asarray(inputs["msg"], dtype=np.float32))
    index = np.asarray(inputs["index"]).astype(np.int64)
    t = np.asarray(inputs["t"], dtype=np.float32)
    W = np.asarray(inputs["W"], dtype=np.float32)
    b = np.asarray(inputs["b"], dtype=np.float32)
    dim_size = int(inputs["dim_size"])

    in_maps, node_ids, key = _prepare(msg, index, t, dim_size, W, b)
    n0, n1 = key
    if key not in _NC_CACHE:
        _NC_CACHE[key] = _build_nc(n0, n1)
    nc = _NC_CACHE[key]

    res = run_bass_kernel_spmd(nc, in_maps, list(range(N_CORES)),
                               trace=trace, **run_kwargs)

    hidden = np.zeros((dim_size, DIM), np.float32)
    for c in range(N_CORES):
        r = res.results[c]
        o = np.concatenate([np.asarray(r["o0"], np.float32),
                            np.asarray(r["o1"], np.float32)], axis=0)
        hc = o.T                                      # [SLOTS, 256]
        valid = node_ids[c] >= 0
        hidden[node_ids[c][valid]] = hc[valid]
    return hidden, res


def kernel(**inputs) -> np.ndarray:
    hidden, _ = _run(inputs, trace=False)
    return hidden


# revision 20
# speedup vs baseline: 1.0349x; 1.0349x over previous
"""GRU-style segmented-scan aggregator for Trainium2 (8 NeuronCores).

Reference computes, per node n with messages m_1..m_L sorted by time t:
    h <- W @ (m + h) + b   starting from h = 0
and returns the final h per node (zeros for empty nodes).

Because every step uses the SAME matrix W, the final state has the closed
form (h_0 = 0):
    h = sum_{k=0}^{L-1} W^{k+1} m_{(L-1-k)}  +  S_L b,   S_L = sum_{p<L} W^p
i.e. the k-th message FROM THE END is hit by W^{k+1}.  This turns the
sequential scan into independent batched matmuls against precomputed powers
of W -- ideal for the PE array.

Tricks that shrink the device work:
  * Two-level blocking (B = 16): split k = q*B + r, so W^{k+1} = W^{qB} W^{r+1}.
    Each device accumulates block partials c_q with only W^1..W^B streamed,
    then folds the q=1 block with one extra matmul h += W^B c_1.  Messages at
    k >= 2B (~0.02%) fold into the host term.  The q=1 step that needs W^B
    is hoisted to stream block B-2 (which carries a copy of W^B), so c_1 is
    complete one block early and its bf16 drain overlaps the last block.
  * Bias fold: h = sum W^{k+1} m'_k with m'_last = m_last + W^{-1} S_L b,
    so no bias pass exists on device (safe here: ||W^{-1} S_L b|| <= ~4).

Everything streams in bf16 (1 PE cycle/column vs 4 for fp32; half the HBM
bytes), accumulates in fp32 PSUM.

Per-core schedule (SPMD over 8 cores; nodes sorted by message count desc and
dealt round-robin, <= 1024 slots/core):
  * PSUM: per 128-feature chunk i, accumulators p0a (slots 0-511), p0b
    (slots 512-1023), p1 (q=1 block, <=512 slots).  One bank each.
  * step r: lhsT = W^{r+1}; rhs columns = r-th-from-block-end messages.
    n0_r / n1_r shrink as segments exhaust -- ~0% padding.
  * stream blocks are CONTIGUOUS dram tensors; two DMA paths run ahead of
    the PE freely (zero SBUF reuse, so DMA never blocks on compute):
      - hi_r = [w_{r+1} | q1 msgs | q0 feature-hi] on the sync HWDGE queue
        (lane-credit recycling waits are legal single waits for loads),
      - q0 feature-lo packed into 6 SWDGE super-blocks, which leaves the
        2 output stores fresh SWDGE lanes (a recycled lane would need
        credit + data = 2 waits; this walrus build allows only one).
  * tail: c_1 -> bf16 (DVE) during block B-1; outer matmul W^B c_1; PSUM
    drains run as two per-chunk chains (chunk0 on DVE, chunk1 on ACT) into
    one [128, 1024] tile each, stored by one SWDGE DMA each.

Host does the (cheap) marshalling: lexsort by (index, t), gather into the
(r, q)-major column layout, precompute W powers in fp64, scatter back.
"""

import numpy as np
import ml_dtypes

import concourse.bass as bass
import concourse.mybir as mybir
from concourse import tile
from concourse.bass_utils import run_bass_kernel_spmd
import bass_rust

_N_PROCS = 27

BF16 = ml_dtypes.bfloat16


class _SplitDrainTC(tile.TileContext):
    """TileContext whose kernel-tail drain is split into one drain per proc.

    The walrus build in this container rejects instructions carrying more
    than one sync wait; the stock tail drain waits on every proc at once.
    Emitting a chain of drains, each waiting on a single semaphore, is
    semantically identical (all procs quiesced before the exit barrier).
    """

    def _drain_and_barrier(self, tick_clock, wait_clock):
        gc = tick_clock.global_clock
        for p in range(_N_PROCS):
            if gc[p] <= 0:
                continue
            d = self.nc.sync.drain()
            vc = bass_rust.VectorClock(
                [gc[q] if q == p else 0 for q in range(_N_PROCS)])
            wait_clock.add_sem_waits(d.ins, bass_rust.ScopedClock({None: vc}))
        assert self.sems is not None
        popped = self.nc._tile_sem_poison_stack.pop()
        assert popped is self._sem_poison
        # One barrier, no per-sem clear: the compiler's kernel epilogue
        # zeroes every semaphore after this block anyway, so the stock
        # clear_and_free + second barrier only add exit latency.
        self.nc.all_engine_barrier()

N_CORES = 8
DIM = 256
SLOTS = 1024  # node slots per core
BLK = 16      # power-blocking factor B


def _q1_steps(R1):
    """q=1 steps per stream block: step r at block r, except a step at
    block BLK-1 is hoisted to block BLK-2 (with a W^B weight copy) so c_1
    finishes a block early."""
    steps = [[] for _ in range(BLK)]
    for s in range(R1):
        steps[min(s, BLK - 2)].append(s)
    return steps


N_LO_SWDGE = 4  # lo blocks issued on SWDGE (4 + 2 stores = 6 of 8 fresh lanes)


_NC_CACHE: dict = {}


def _build_nc(n0: tuple, n1: tuple):
    """Build the Bass program for one core (shared by all 8 via SPMD)."""
    f32 = mybir.dt.float32
    bf16 = mybir.dt.bfloat16
    nc = bass.Bass()

    R0 = len(n0)
    R1 = len(n1)
    nq1 = n1[0] if R1 else 0
    rb_last = max((r for r in range(R0) if n0[r] > 512), default=-1)
    q1s = _q1_steps(R1)

    def hi_width(r):
        w = 512 + 2 * sum(n1[s] for s in q1s[r])
        if any(s == BLK - 1 for s in q1s[r]):
            w += 512                       # W^B copy for the hoisted step
        return w

    wq0 = nc.dram_tensor("wq0", [128, hi_width(0)], bf16,
                         kind="ExternalInput")
    his = [nc.dram_tensor(f"hi{r}",
                          [128, (hi_width(r) if r else 0) + n0[r]],
                          bf16, kind="ExternalInput") for r in range(R0)]
    los = [nc.dram_tensor(f"lo{r}", [128, n0[r]], bf16, kind="ExternalInput")
           for r in range(R0)]
    outs = [nc.dram_tensor(f"o{i}", [128, SLOTS], bf16, kind="ExternalOutput")
            for i in range(2)]

    with _SplitDrainTC(nc) as tc:
        with (
            tc.tile_pool(name="m", bufs=1) as mpool,
            tc.tile_pool(name="misc", bufs=1) as miscpool,
            tc.tile_pool(name="ps", bufs=1, space="PSUM") as pspool,
        ):
            p0a = [pspool.tile([128, 512], f32, tag=f"p0a{i}", name=f"p0a{i}")
                   for i in range(2)]
            p0b = [pspool.tile([128, 512], f32, tag=f"p0b{i}", name=f"p0b{i}")
                   for i in range(2)]
            p1 = [pspool.tile([128, 512], f32, tag=f"p1{j}", name=f"p1{j}")
                  for j in range(2)] if R1 else None

            wblk = None          # tile+offset holding W^B for the outer fold
            for r in range(R0):
                a = n0[r]
                hw = hi_width(r)
                if r == 0:
                    tw = mpool.tile([128, hw], bf16, tag="tw0", name="tw0")
                    nc.sync.dma_start(tw[:], wq0[:])
                    th = mpool.tile([128, a], bf16, tag="th0", name="th0")
                    nc.sync.dma_start(th[:], his[0][:])
                    hoff = 0
                else:
                    tw = th = mpool.tile([128, hw + a], bf16,
                                         tag=f"th{r}", name=f"th{r}")
                    nc.sync.dma_start(th[:], his[r][:])
                    hoff = hw
                tg = mpool.tile([128, a], bf16, tag=f"tl{r}", name=f"tl{r}")
                goff = 0
                # first few lo blocks ride the (otherwise idle) SWDGE path so
                # the cold-start feed keeps up; the rest stay on the sync
                # queue in need order
                if r < N_LO_SWDGE:
                    nc.gpsimd.dma_start(tg[:], los[r][:])
                else:
                    nc.sync.dma_start(tg[:], los[r][:])
                if r == BLK - 1 and R1 and wblk is None:
                    wblk = (tw, 0)
                # q1 message offsets within tw
                moff = 512 + (512 if any(s == BLK - 1 for s in q1s[r]) else 0)
                q1list = []
                for s in (q1s[r] if r < len(q1s) else []):
                    woff = 512 if s == BLK - 1 else 0
                    if s == BLK - 1:
                        wblk = (tw, 512)
                    q1list.append((s, n1[s], woff, moff))
                    moff += 2 * n1[s]
                for i in range(2):          # output feature chunk
                    for j in range(2):      # contraction chunk
                        wt = tw[:, j * 256 + i * 128: j * 256 + (i + 1) * 128]
                        src, boff = (th, hoff) if j == 0 else (tg, goff)
                        nc.tensor.matmul(
                            p0a[i][:, 0:min(a, 512)], wt,
                            src[:, boff: boff + min(a, 512)],
                            start=(r == 0 and j == 0), stop=False,
                            skip_group_check=True,
                        )
                        if a > 512:
                            nc.tensor.matmul(
                                p0b[i][:, 0:a - 512], wt,
                                src[:, boff + 512: boff + a],
                                start=(r == 0 and j == 0),
                                stop=(r == rb_last and i == 1 and j == 1),
                                skip_group_check=True,
                            )
                        for s, cs, woff, mo in q1list:
                            wts = tw[:, woff + j * 256 + i * 128:
                                     woff + j * 256 + (i + 1) * 128]
                            nc.tensor.matmul(
                                p1[i][:, 0:cs], wts,
                                tw[:, mo + j * cs: mo + (j + 1) * cs],
                                start=(r == 0 and j == 0), stop=False,
                                skip_group_check=True,
                            )
                # c_1 -> bf16 right after its last inner step (block B-2):
                # the DVE drain overlaps block B-1 on the PE
                if R1 and r == BLK - 2:
                    c1 = []
                    for j in range(2):
                        t = miscpool.tile([128, nq1], bf16, tag=f"c1{j}",
                                          name=f"c1{j}")
                        nc.vector.tensor_copy(t[:], p1[j][:, 0:nq1])
                        c1.append(t)

            if R1 and R0 < BLK:              # no hoist happened; drain now
                c1 = []
                for j in range(2):
                    t = miscpool.tile([128, nq1], bf16, tag=f"c1{j}",
                                      name=f"c1{j}")
                    nc.vector.tensor_copy(t[:], p1[j][:, 0:nq1])
                    c1.append(t)

            # outer fold q=1:  p0a += (W^B).T.T @ c1
            if R1:
                assert wblk is not None
                wtile, wo = wblk
                for i in range(2):
                    for j in range(2):
                        wt = wtile[:, wo + j * 256 + i * 128:
                                   wo + j * 256 + (i + 1) * 128]
                        nc.tensor.matmul(
                            p0a[i][:, 0:nq1], wt, c1[j][:],
                            start=False, stop=(i == 1 and j == 1),
                            skip_group_check=True,
                        )

            # drain PSUM -> bf16 SBUF as two per-chunk chains (chunk 0 on
            # DVE, chunk 1 on ACT) into one tile each, then one SWDGE store
            # per chunk (fresh DMASW lanes -> single producer wait)
            ot = [miscpool.tile([128, SLOTS], bf16, tag=f"ot{i}", name=f"ot{i}")
                  for i in range(2)]
            nc.vector.tensor_copy(ot[0][:, 512:1024], p0b[0][:])
            nc.scalar.copy(ot[1][:, 512:1024], p0b[1][:])
            nc.vector.tensor_copy(ot[0][:, 0:512], p0a[0][:])
            nc.scalar.copy(ot[1][:, 0:512], p0a[1][:])
            for i in range(2):
                nc.gpsimd.dma_start(outs[i][:], ot[i][:])
    return nc


def _prepare(msg, index, t, dim_size, W, b):
    """Host-side marshalling. Returns (in_maps, node_ids, schedule key)."""
    E, D = msg.shape
    counts = np.bincount(index, minlength=dim_size)
    order = np.lexsort((t, index))            # stable: primary index, secondary t
    msg_sorted = msg[order]                   # [E, D] grouped by node, t-ascending
    seg_starts = np.zeros(dim_size, np.int64)
    seg_starts[1:] = np.cumsum(counts)[:-1]

    nodesort = np.argsort(-counts, kind="stable")
    nz = nodesort[counts[nodesort] > 0]
    per_core = -(-len(nz) // N_CORES)
    assert per_core <= SLOTS, f"too many nodes per core: {per_core}"

    node_ids = np.full((N_CORES, SLOTS), -1, np.int64)
    for c in range(N_CORES):
        ids = nz[c::N_CORES]
        node_ids[c, :len(ids)] = ids
    cc = np.where(node_ids >= 0, counts[np.maximum(node_ids, 0)], 0)  # [8, SLOTS]

    Lmax = int(cc.max())
    K0 = min(Lmax, 2 * BLK)
    R0 = min(BLK, K0)
    R1 = max(0, K0 - BLK)
    n0 = tuple(int((cc > r).sum(axis=1).max()) for r in range(R0))
    n1 = tuple(int((cc > BLK + r).sum(axis=1).max()) for r in range(R1))
    assert (not n1) or n1[0] <= 512, f"q1 region too wide: {n1[0]}"
    q1s = _q1_steps(R1)

    def slot_pos(qb, r, nslots):
        nid = node_ids[:, :nslots]
        ck = cc[:, :nslots]
        active = (qb + r) < ck
        pos = seg_starts[np.maximum(nid, 0)] + ck - 1 - (qb + r)
        return np.where(active, pos, -1)

    # weights: powers of W in fp64, stored transposed (lhsT chunks)
    Wd = W.astype(np.float64)
    bd = b.astype(np.float64)
    wfull = np.empty((128, R0 * 512), np.float64)
    s_table = np.zeros((Lmax + 1, D), np.float64)   # s_p = S_p b
    Wpows = []                                      # W^{k+1}, k = 0..Lmax-1
    P = Wd.copy()
    for k in range(Lmax):
        if k < R0:
            WT = P.T
            wfull[:, k * 512:k * 512 + 256] = WT[:128, :]
            wfull[:, k * 512 + 256:(k + 1) * 512] = WT[128:, :]
        Wpows.append(P)
        s_table[k + 1] = Wd @ s_table[k] + bd
        P = P @ Wd
    wfull16 = wfull.astype(BF16)

    # bias fold into the last message (+ host-folded k >= K0 tail)
    WiS = np.linalg.solve(Wd, s_table.T).T          # [Lmax+1, 256]
    fold = WiS[cc]                                   # [8, SLOTS, 256] fp64
    for k in range(K0, Lmax):
        act = k < cc
        cs, ss = np.nonzero(act)
        pos = seg_starts[node_ids[cs, ss]] + cc[cs, ss] - 1 - k
        Wk = Wpows[k - 1] if k >= 1 else np.eye(D)   # W^k
        fold[cs, ss] += msg_sorted[pos].astype(np.float64) @ Wk.T

    def gathered16(qb, r, nslots, c):
        ri = slot_pos(qb, r, nslots)[c]
        if qb == 0 and r == 0:
            Mg = (msg_sorted[np.maximum(ri, 0)].astype(np.float64)
                  + fold[c, :nslots]).astype(BF16)
        else:
            Mg = msg16[np.maximum(ri, 0)]
        Mg[ri < 0] = 0.0
        return Mg                                     # [nslots, 256]

    msg16 = msg_sorted.astype(BF16)
    in_maps = [dict() for _ in range(N_CORES)]
    for r in range(R0):
        a = n0[r]
        hoisted = any(s == BLK - 1 for s in q1s[r])
        hw = 512 + (512 if hoisted else 0) + 2 * sum(n1[s] for s in q1s[r])
        for c in range(N_CORES):
            blk = np.zeros((128, hw + a), BF16)
            blk[:, 0:512] = wfull16[:, r * 512:(r + 1) * 512]
            mo = 512
            if hoisted:
                blk[:, 512:1024] = wfull16[:, (BLK - 1) * 512:BLK * 512]
                mo = 1024
            for s in q1s[r]:
                cs = n1[s]
                Mg1 = gathered16(BLK, s, cs, c)
                blk[:, mo:mo + cs] = Mg1[:, :128].T
                blk[:, mo + cs:mo + 2 * cs] = Mg1[:, 128:].T
                mo += 2 * cs
            Mg = gathered16(0, r, a, c)
            blk[:, hw:] = Mg[:, :128].T
            if r == 0:
                in_maps[c]["wq0"] = np.ascontiguousarray(blk[:, :hw])
                in_maps[c]["hi0"] = np.ascontiguousarray(blk[:, hw:])
            else:
                in_maps[c][f"hi{r}"] = blk
            in_maps[c][f"lo{r}"] = np.ascontiguousarray(Mg[:, 128:].T)
    return in_maps, node_ids, (n0, n1)


def _run(inputs: dict, trace: bool = False, **run_kwargs):
    msg = np.ascontiguousarray(np.asarray(inputs["msg"], dtype=np.float32))
    index = np.asarray(inputs["index"]).astype(np.int64)
    t = np.asarray(inputs["t"], dtype=np.float32)
    W = np.asarray(inputs["W"], dtype=np.float32)
    b = np.asarray(inputs["b"], dtype=np.float32)
    dim_size = int(inputs["dim_size"])

    in_maps, node_ids, key = _prepare(msg, index, t, dim_size, W, b)
    n0, n1 = key
    if key not in _NC_CACHE:
        _NC_CACHE[key] = _build_nc(n0, n1)
    nc = _NC_CACHE[key]

    res = run_bass_kernel_spmd(nc, in_maps, list(range(N_CORES)),
                               trace=trace, **run_kwargs)

    hidden = np.zeros((dim_size, DIM), np.float32)
    for c in range(N_CORES):
        r = res.results[c]
        o = np.concatenate([np.asarray(r["o0"], np.float32),
                            np.asarray(r["o1"], np.float32)], axis=0)
        hc = o.T                                      # [SLOTS, 256]
        valid = node_ids[c] >= 0
        hidden[node_ids[c][valid]] = hc[valid]
    return hidden, res


def kernel(**inputs) -> np.ndarray:
    hidden, _ = _run(inputs, trace=False)
    return hidden
